# revision 38
# baseline (speedup 1.0000x reference)
"""DeepseekV2 MoE layer on 8 Trainium2 NeuronCores.

Strategy (expert-parallel, matching the sharding hint):
  - Host: gate (softmax + top-6) in float64, stable dispatch by expert —
    bit-identical routing to the fp32 reference.
  - Device, per core c (SPMD, one program): 4 experts' GLU MLPs (fp8e4m3
    weights/activations, DoubleRow matmuls) on the gathered token buffer
    (capacity 128 >= observed max count 117), plus a 1/8 tensor-parallel
    shard of the shared-expert GLU in fp16 (FS 2816 -> 352, tight pack).
  - Host: weighted scatter-add combine + sum of shared partials (fp64).

Performance model (measured):
  - PE floor ~111 us/core (routed fp8-DR 70 + shared fp16 37 + transposes 4,
    all ~1 column/cycle @ 2 GHz).  DMA floor ~105 us (45.2 MB/core at the
    ~430 GB/s per-core streaming rate; chip aggregate saturates ~3.2 TB/s).
  - The previous 164-181 us came from coupling stalls: PSUM hit 8/8 banks
    during shared stage 1 (4 parallel accumulators), DMA-trigger engines
    (sync+scalar HWDGE) blocked head-of-line on buffer-recycle semaphores,
    and a 13-18 us serial tail (last expert s1->mul->transpose->s2->store).

Schedule in this version (measured ~159 us max-core / ~149 mean, vs
164-183 for the previous kernel):
  - Weight DMAs stream in 4-ho stage-1 tiles (ws1 pool 12 slots) and
    4/4/3-fc stage-2 tiles (ws2 pool 6 slots).  Triggers for expert e+1
    are drip-fed from inside expert e's body right AFTER each silu batch
    (~2-3 transfers per ring per phase).  Two hard-won rules:
      (a) a DMA trigger occupies its HWDGE engine until the ring has
          space (~2 outstanding transfers), so any batch of triggers
          queued on the scalar ring ahead of silus delays them at DMA
          pace (a 20+ us convoy);
      (b) pool depths are sized so every trigger's buffer-recycle
          semaphore is already satisfied when the engine reaches it.
    Finer (4-ho) tiles halve the stream-lag whiplash at the ws1 pool
    limit near expert boundaries.
  - Shared-expert compute is cut into 12 quanta (4 gate, 4 up+transpose,
    4 down+store) used as PE filler at expert phase boundaries; shared
    stage-1 accumulates per-token-block sequentially (1 PSUM bank at a
    time instead of 4, which used to peg PSUM at 8/8 banks).
  - Expert 3's stage-2 runs hn-major on its last weight tile so each
    psy's cast+store pipelines with the remaining matmuls (short tail).
  - ye outputs are stored as fp8e4m3 at scale 64 (adds 0.41% rel err in
    isolation; total 1.09% vs the 2% gate), shared weights tight-packed
    (352, no 384 pad): 45.2 MB/core vs 46.7 baseline.
"""

import os
import numpy as np

T, H, E, K = 512, 2048, 32, 6
F, FS = 1408, 2816
NCORES = 8
EPC = E // NCORES          # experts per core = 4
CAPD = 128                 # device per-expert capacity (max observed count 117)
CAP_REF = 160              # reference capacity (for drop semantics; no drops here)
HO = H // 128              # 16
FO = F // 128              # 11
TOK = T // 128             # 4
FSH = FS // NCORES         # 352 shared-intermediate shard (tight, no pad)
JT = [(0, 512), (512, 512), (1024, 384)]   # stage-1 f tiles
FS_CHUNKS = [(0, 128), (128, 128), (256, 96)]  # shared-intermediate chunks
S2_TILES = [(0, 4), (4, 4), (8, 3)]            # stage-2 f-chunk tiles

SILU_SCALE = 1.0 / 256.0   # fp8 psum -> h scale (w_scale 256 undone)
HT_SCALE = 1.0 / 16.0      # h -> hT fp8 scale
W_SCALE = 256.0            # expert weight quantization scale
W_CLIP = 224.0
YE_STORE_SCALE = 1.0 / 64.0   # psy (= 4096*y) -> fp8 store (= 64*y)
YE_UNSCALE = 64.0             # host divides stored ye by this
# NOTE: platform float8e4 is IEEE-style e4m3 with max 240 (not e4m3fn/448);
# max |64*y| ~ 130 leaves 1.8x headroom.

LAST_RESULTS = {}
_NC_CACHE = {}


def _build_nc():
    import concourse.tile as tile
    from concourse import mybir, bacc
    from concourse.masks import make_identity

    f32 = mybir.dt.float32
    f16 = mybir.dt.float16
    fp8 = mybir.dt.float8e4
    dr = mybir.MatmulPerfMode.DoubleRow

    nc = bacc.Bacc(None, target_bir_lowering=False, debug=False)

    xeT = nc.dram_tensor("xeT", [128, HO, EPC * CAPD], fp8, kind="ExternalInput")
    wgu = nc.dram_tensor("wgu", [EPC, 2, 128, HO, F], fp8, kind="ExternalInput")
    wd = nc.dram_tensor("wd", [EPC, 128, FO, H], fp8, kind="ExternalInput")
    xTr = nc.dram_tensor("xTr", [128, TOK, HO, 128], f16, kind="ExternalInput")
    wsgu = nc.dram_tensor("wsgu", [2, 128, HO, FSH], f16, kind="ExternalInput")
    wsd = nc.dram_tensor("wsd", [FSH, H], f16, kind="ExternalInput")
    ye = nc.dram_tensor("ye", [EPC, CAPD, H], fp8, kind="ExternalOutput")
    part = nc.dram_tensor("part", [TOK, 128, H], f16, kind="ExternalOutput")

    # Pool depths are sized so every weight-DMA trigger's buffer-recycle
    # semaphore is already satisfied when the trigger engine reaches it
    # (waves are emitted one expert ahead; ws1 holds ~1.75 experts of
    # stage-1 tiles, ws2 exactly 2 experts of stage-2 tiles).  A trigger
    # that waits head-of-line on the scalar ring delays the silus behind
    # it, which PE needs — that convoy was worth 20+ us/core.
    ws1_bufs = int(os.environ.get("KERNEL_WS1_BUFS", "12"))
    ws2_bufs = int(os.environ.get("KERNEL_WS2_BUFS", "6"))

    # Both HWDGE rings (sync + scalar) carry traffic round-robin.
    dma_engines = [nc.sync, nc.scalar]
    dma_i = [0]

    def dma(out_ap, in_ap):
        eng = dma_engines[dma_i[0] % 2]
        dma_i[0] += 1
        eng.dma_start(out_ap, in_ap)

    with tile.TileContext(nc) as tc:
        with (
            tc.tile_pool(name="res", bufs=2) as sb_res,
            tc.tile_pool(name="const", bufs=1) as sb_const,
            tc.tile_pool(name="ws1", bufs=ws1_bufs) as sb_w1,
            tc.tile_pool(name="ws2", bufs=ws2_bufs) as sb_w2,
            tc.tile_pool(name="wsh", bufs=4) as sb_wsh,
            tc.tile_pool(name="act", bufs=2) as sb_act,
            tc.tile_pool(name="oye", bufs=1) as sb_oye,
            tc.tile_pool(name="opart", bufs=1) as sb_opart,
            tc.tile_pool(name="acc", bufs=4, space="PSUM") as ps_acc,
            tc.tile_pool(name="py", bufs=4, space="PSUM") as ps_y,
        ):
            ident = sb_const.tile([128, 128], f16, tag="ident")
            make_identity(nc, ident)

            state = {}
            s1_tiles = {}
            s2_tiles = {}

            def issue_s1(e, proj, t):
                wt = sb_w1.tile([128, 4, F], fp8, tag="ws1",
                                name=f"w{e}_{proj}_{t}")
                dma(wt[:], wgu[e, proj, :, 4 * t: 4 * t + 4, :])
                s1_tiles[(e, proj, t)] = wt

            def issue_s2(e):
                lst = []
                for ti, (f0, fw) in enumerate(S2_TILES):
                    wt = sb_w2.tile([128, 4, H], fp8, tag="ws2",
                                    name=f"wd{e}_{ti}")
                    dma(wt[:, :fw, :], wd[e, :, f0: f0 + fw, :])
                    lst.append((f0, fw, wt))
                s2_tiles[e] = lst

            # ---------------- PRE: just enough to start e0 ----------------
            # Trigger cadence rule: a DMA trigger occupies its HWDGE engine
            # until the ring has space (~2 outstanding transfers), so any
            # batch of triggers queued on the scalar ring ahead of silus
            # delays them at DMA pace.  Keep <= ~5 transfers per ring ahead
            # of the next silu batch; later triggers are drip-fed from
            # inside the expert bodies right AFTER each silu batch.
            xeT_sb = sb_res.tile([128, HO, EPC * CAPD], fp8, tag="res",
                                 name="xeT_sb")
            xTr_sb = sb_res.tile([128, TOK, HO, 128], f16, tag="res",
                                 name="xTr_sb")
            issue_s1(0, 0, 0)
            dma(xeT_sb[:, 0:8, :], xeT[:, 0:8, :])
            issue_s1(0, 0, 1)
            issue_s1(0, 0, 2)
            wsg_tiles = []
            for t_ in range(2):
                wt = sb_wsh.tile([128, 8, FSH], f16, tag="wsh", name=f"wsg{t_}")
                dma(wt[:], wsgu[0, :, 8 * t_: 8 * t_ + 8, :])
                wsg_tiles.append(wt)
            state["wsg"] = wsg_tiles
            issue_s1(0, 0, 3)
            dma(xTr_sb[:, 0, :, :], xTr[:, 0, :, :])
            dma(xeT_sb[:, 8:16, :], xeT[:, 8:16, :])
            issue_s1(0, 1, 0)
            issue_s1(0, 1, 1)
            dma(xTr_sb[:, 1, :, :], xTr[:, 1, :, :])
            issue_s1(0, 1, 2)
            issue_s1(0, 1, 3)

            def issue_wsu():
                wsu_tiles = []
                for t_ in range(2):
                    wt = sb_wsh.tile([128, 8, FSH], f16, tag="wsh",
                                     name=f"wsu{t_}")
                    dma(wt[:], wsgu[1, :, 8 * t_: 8 * t_ + 8, :])
                    wsu_tiles.append(wt)
                state["wsu"] = wsu_tiles

            def issue_wsd():
                wsd_sb = sb_const.tile([128, 3, H], f16, tag="wsd",
                                       name="wsd_sb")
                dma(wsd_sb[:, 0, :], wsd[0:128, :])
                dma(wsd_sb[:, 1, :], wsd[128:256, :])
                dma(wsd_sb[:96, 2, :], wsd[256:352, :])
                state["wsd"] = wsd_sb

            def issue_xtr_tail():
                dma(xTr_sb[:, 2, :, :], xTr[:, 2, :, :])
                dma(xTr_sb[:, 3, :, :], xTr[:, 3, :, :])

            hs_all = sb_const.tile([128, TOK, FSH], f16, tag="hs", name="hs_all")
            hsT_all = sb_const.tile([128, TOK, 3, 128], f16, tag="hsT",
                                    name="hsT_all")

            # ---------------- shared-expert quanta ----------------
            def q_gate(tc_):
                def run():
                    psg = ps_acc.tile([128, FSH], f32, tag="acc",
                                      name=f"psg_{tc_}")
                    for t_ in range(2):
                        wt = state["wsg"][t_]
                        for hh in range(8):
                            ho = 8 * t_ + hh
                            nc.tensor.matmul(
                                psg[:],
                                xTr_sb[:, tc_, ho, :],
                                wt[:, hh, :],
                                start=(ho == 0),
                                stop=(ho == HO - 1),
                            )
                    nc.scalar.activation(
                        hs_all[:, tc_, :], psg[:],
                        mybir.ActivationFunctionType.Silu,
                    )
                return run

            def q_up(tc_):
                def run():
                    psu = ps_acc.tile([128, FSH], f32, tag="acc",
                                      name=f"psu_{tc_}")
                    for t_ in range(2):
                        wt = state["wsu"][t_]
                        for hh in range(8):
                            ho = 8 * t_ + hh
                            nc.tensor.matmul(
                                psu[:],
                                xTr_sb[:, tc_, ho, :],
                                wt[:, hh, :],
                                start=(ho == 0),
                                stop=(ho == HO - 1),
                            )
                    nc.vector.tensor_mul(
                        out=hs_all[:, tc_, :],
                        in0=hs_all[:, tc_, :],
                        in1=psu[:],
                    )
                    for c, (c0, cw) in enumerate(FS_CHUNKS):
                        pt = ps_y.tile([128, 512], f16, tag="py",
                                       name=f"pts_{tc_}_{c}")
                        nc.tensor.transpose(
                            pt[:cw, :128], hs_all[:, tc_, c0: c0 + cw], ident[:]
                        )
                        nc.vector.tensor_copy(
                            hsT_all[:cw, tc_, c, :], pt[:cw, :128]
                        )
                return run

            def q_down(tc_):
                def run():
                    wsd_sb = state["wsd"]
                    part_sb = sb_opart.tile([128, H], f16, tag="opart",
                                            name=f"part_{tc_}")
                    for hn in range(4):
                        psy = ps_y.tile([128, 512], f32, tag="py",
                                        name=f"pys_{tc_}_{hn}")
                        for c, (c0, cw) in enumerate(FS_CHUNKS):
                            nc.tensor.matmul(
                                psy[:],
                                hsT_all[:cw, tc_, c, :],
                                wsd_sb[:cw, c, hn * 512: (hn + 1) * 512],
                                start=(c == 0),
                                stop=(c == 2),
                            )
                        nc.vector.tensor_copy(
                            part_sb[:, hn * 512: (hn + 1) * 512], psy[:]
                        )
                    dma(part[tc_], part_sb[:])
                return run

            # ---------------- routed expert ----------------
            def expert(e, q_p0=(), q_p1=(), q_tr=(), split_out=False):
                esl = slice(e * CAPD, (e + 1) * CAPD)
                h_sb = sb_act.tile([128, F], f16, tag="h", name=f"h_{e}")
                for proj in range(2):
                    ps_j = [
                        ps_acc.tile([128, jw], f32, tag="acc",
                                    name=f"ps_{e}_{proj}_{j}")
                        for j, (j0, jw) in enumerate(JT)
                    ]
                    for t_ in range(4):
                        wt = s1_tiles.pop((e, proj, t_))
                        for hp in range(2):
                            ho = 4 * t_ + 2 * hp
                            for j, (j0, jw) in enumerate(JT):
                                nc.tensor.matmul(
                                    ps_j[j][:],
                                    xeT_sb[:, ho: ho + 2, esl],
                                    wt[:, 2 * hp: 2 * hp + 2, j0: j0 + jw],
                                    start=(ho == 0),
                                    stop=(ho == HO - 2),
                                    perf_mode=dr,
                                )
                    if proj == 0:
                        for j, (j0, jw) in enumerate(JT):
                            nc.scalar.activation(
                                h_sb[:, j0: j0 + jw],
                                ps_j[j][:],
                                mybir.ActivationFunctionType.Silu,
                                scale=SILU_SCALE,
                            )
                        for fn in q_p0:
                            fn()
                    else:
                        for j, (j0, jw) in enumerate(JT):
                            nc.vector.tensor_mul(
                                out=h_sb[:, j0: j0 + jw],
                                in0=h_sb[:, j0: j0 + jw],
                                in1=ps_j[j][:],
                            )
                        for fn in q_p1:
                            fn()

                hT_sb = sb_act.tile([128, FO, CAPD], fp8, tag="hT",
                                    name=f"hT_{e}")
                for fc in range(FO):
                    pt = ps_y.tile([128, 512], f16, tag="py",
                                   name=f"pt_{e}_{fc}")
                    nc.tensor.transpose(
                        pt[:, :128], h_sb[:, fc * 128: (fc + 1) * 128], ident[:]
                    )
                    nc.vector.tensor_scalar_mul(
                        hT_sb[:, fc, :], pt[:, :128], HT_SCALE
                    )
                for fn in q_tr:
                    fn()

                psy = [
                    ps_y.tile([128, 512], f32, tag="py", name=f"py_{e}_{hn}")
                    for hn in range(4)
                ]
                tiles = s2_tiles.pop(e)
                ye_sb = sb_oye.tile([128, H], fp8, tag="oye", name=f"ye_{e}")
                # last tile handled hn-major on the final expert so each
                # psy's cast+store pipelines with the remaining matmuls
                body, last = (tiles[:-1], tiles[-1]) if split_out else (tiles, None)
                for f0, fw, wt in body:
                    ff = 0
                    while ff < fw:
                        fc = f0 + ff
                        if ff + 1 < fw:
                            for hn in range(4):
                                nc.tensor.matmul(
                                    psy[hn][:],
                                    hT_sb[:, fc: fc + 2, :],
                                    wt[:, ff: ff + 2,
                                       hn * 512: (hn + 1) * 512],
                                    start=(fc == 0),
                                    stop=(fc + 2 == FO),
                                    perf_mode=dr,
                                )
                            ff += 2
                        else:
                            for hn in range(4):
                                nc.tensor.matmul(
                                    psy[hn][:],
                                    hT_sb[:, fc, :],
                                    wt[:, ff, hn * 512: (hn + 1) * 512],
                                    start=(fc == 0),
                                    stop=(fc == FO - 1),
                                )
                            ff += 1
                # casts alternate scalar (Copy activation with scale) and
                # vector so the two engines drain the 4 psy banks in
                # parallel — matters most on the last expert's tail
                def ye_cast(hn):
                    hsl = slice(hn * 512, (hn + 1) * 512)
                    if hn % 2 == 0:
                        nc.scalar.activation(
                            ye_sb[:, hsl], psy[hn][:],
                            mybir.ActivationFunctionType.Copy,
                            scale=YE_STORE_SCALE,
                        )
                    else:
                        nc.vector.tensor_scalar_mul(
                            ye_sb[:, hsl], psy[hn][:], YE_STORE_SCALE,
                        )
                    return hsl

                if split_out:
                    f0, fw, wt = last
                    for hn in range(4):
                        hsl = slice(hn * 512, (hn + 1) * 512)
                        nc.tensor.matmul(
                            psy[hn][:], hT_sb[:, f0: f0 + 2, :],
                            wt[:, 0:2, hsl], start=False, stop=False,
                            perf_mode=dr,
                        )
                        nc.tensor.matmul(
                            psy[hn][:], hT_sb[:, f0 + 2, :],
                            wt[:, 2, hsl], start=False, stop=True,
                        )
                        ye_cast(hn)
                        dma(ye[e][:, hsl], ye_sb[:, hsl])
                else:
                    for hn in range(4):
                        ye_cast(hn)
                    dma(ye[e], ye_sb[:])

            # ---------------- schedule ----------------
            # Each expert body drip-feeds the NEXT expert's weight triggers
            # right after its own silu batches (~3 transfers per ring per
            # phase), so the scalar ring never queues enough pending
            # transfers to delay a silu, and every trigger's buffer-recycle
            # semaphore is already satisfied when issued (pool depths hold
            # 1.5 experts of stage-1 and 2 of stage-2 tiles).
            def s1w(e, proj):
                def run():
                    for t_ in range(4):
                        issue_s1(e, proj, t_)
                return run

            def s2w(e):
                return lambda: issue_s2(e)

            expert(
                0,
                q_p0=[q_gate(0), s2w(0), issue_xtr_tail],
                q_p1=[q_gate(1), s1w(1, 0), lambda: issue_wsu()],
                q_tr=[q_gate(2), s1w(1, 1)],
            )
            expert(
                1,
                q_p0=[q_gate(3), s2w(1), issue_wsd],
                q_p1=[q_up(0), q_up(1), s1w(2, 0)],
                q_tr=[q_up(2), s1w(2, 1)],
            )
            expert(
                2,
                q_p0=[q_up(3), s2w(2)],
                q_p1=[q_down(0), s1w(3, 0)],
                q_tr=[q_down(1), s1w(3, 1)],
            )
            expert(
                3,
                q_p0=[q_down(2), s2w(3)],
                q_p1=[q_down(3)],
                split_out=True,
            )

    nc.finalize()
    return nc


def _get_nc():
    if "nc" not in _NC_CACHE:
        _NC_CACHE["nc"] = _build_nc()
    return _NC_CACHE["nc"]


def _ensure_ntff_hook():
    """Provide antenv.axon_hooks if the image lacks it (profiling only)."""
    try:
        from antenv.axon_hooks import get_axon_ntff_profile_hook  # noqa: F401

        return True
    except ImportError:
        pass
    try:
        import sys
        import types
        import ctypes
        import contextlib

        so_path = "/opt/axon/libaxon_pjrt.so"
        lib = ctypes.CDLL(so_path)
        if not hasattr(lib, "axon_start_nrt_profile"):
            return False
        lib.axon_start_nrt_profile.argtypes = [
            ctypes.POINTER(ctypes.c_int64),
            ctypes.c_size_t,
        ]
        lib.axon_start_nrt_profile.restype = ctypes.c_int64
        lib.axon_stop_nrt_profile.argtypes = [ctypes.c_char_p]
        lib.axon_stop_nrt_profile.restype = ctypes.c_int64

        @contextlib.contextmanager
        def _hook(output_dir, device_ids):
            import jax

            jax.devices()
            if device_ids:
                ids = (ctypes.c_int64 * len(device_ids))(*device_ids)
                rc = lib.axon_start_nrt_profile(ids, len(device_ids))
            else:
                rc = lib.axon_start_nrt_profile(None, 0)
            if rc != 0:
                raise RuntimeError(f"axon_start_nrt_profile rc={rc}")
            try:
                yield
            finally:
                n = lib.axon_stop_nrt_profile(str(output_dir).encode())
                print(f"ntff profile: {n} file(s) -> {output_dir}", file=sys.stderr)

        import antenv

        mod = types.ModuleType("antenv.axon_hooks")
        _holder = {"hook": _hook}
        mod.get_axon_ntff_profile_hook = lambda: _holder["hook"]

        def _set(h):
            _holder["hook"] = h

        mod.set_axon_ntff_profile_hook = _set
        sys.modules["antenv.axon_hooks"] = mod
        antenv.axon_hooks = mod
        return True
    except Exception:
        return False


def kernel(hidden_states, wg, gate_w, up_w, down_w, sg_w, su_w, sd_w):
    from concourse import mybir
    from concourse.bass_utils import run_bass_kernel_spmd

    x = np.asarray(hidden_states, np.float32)
    wg = np.asarray(wg, np.float32)
    gate_w = np.asarray(gate_w, np.float32)
    up_w = np.asarray(up_w, np.float32)
    down_w = np.asarray(down_w, np.float32)
    sg_w = np.asarray(sg_w, np.float32)
    su_w = np.asarray(su_w, np.float32)
    sd_w = np.asarray(sd_w, np.float32)

    # ---- gate: fp64 softmax + greedy top-k (matches fp32 reference routing;
    #      min 6th/7th margin ~2e-5 >> fp32 rounding noise) ----
    logits = x.astype(np.float64) @ wg.astype(np.float64).T
    m = logits.max(axis=-1, keepdims=True)
    es = np.exp(logits - m)
    scores = es / es.sum(axis=-1, keepdims=True)
    topk_idx = np.argsort(-scores, axis=-1, kind="stable")[:, :K]     # [T, K]
    topk_w = np.take_along_axis(scores, topk_idx, axis=-1)            # [T, K]

    # ---- dispatch: stable sort of (t, k) entries by expert ----
    N = T * K
    flat_e = topk_idx.reshape(-1)
    order = np.argsort(flat_e, kind="stable")
    sorted_e = flat_e[order]
    counts = np.bincount(flat_e, minlength=E)
    offsets = np.cumsum(counts) - counts
    pos_sorted = np.arange(N) - offsets[sorted_e]
    pos_flat = np.empty(N, np.int64)
    pos_flat[order] = pos_sorted
    tok_flat = np.arange(N) // K
    # reference drops entries with pos >= CAP_REF (none for this input);
    # device capacity is CAPD
    assert counts.max() <= CAPD, f"expert overflow: {counts.max()} > {CAPD}"

    buf = np.zeros((E, CAPD, H), np.float32)
    buf[flat_e, pos_flat] = x[tok_flat]

    fp8_np = np.dtype(mybir.dt.np(mybir.dt.float8e4))
    f16_np = np.dtype(np.float16)

    def qw(a):  # quantize an expert weight array to fp8 at scale 256
        a = np.clip(a * W_SCALE, -W_CLIP, W_CLIP)
        return np.ascontiguousarray(a).astype(fp8_np)

    def prep_stage1_w(w_t):  # w_t: [H, Fdim] -> [128, H//128, Fdim]
        fdim = w_t.shape[1]
        return np.ascontiguousarray(
            w_t.reshape(HO, 128, fdim).transpose(1, 0, 2)
        )

    xTr_np = np.ascontiguousarray(
        x.reshape(TOK, 128, HO, 128).transpose(3, 0, 2, 1)
    ).astype(f16_np)

    in_maps = []
    for c in range(NCORES):
        es0 = c * EPC
        xe_core = buf[es0: es0 + EPC].reshape(EPC * CAPD, H)  # [512, H]
        xeT_np = np.ascontiguousarray(
            xe_core.T.reshape(HO, 128, EPC * CAPD).transpose(1, 0, 2)
        ).astype(fp8_np)

        wgu_np = np.empty((EPC, 2, 128, HO, F), fp8_np)
        wd_np = np.empty((EPC, 128, FO, H), fp8_np)
        for el in range(EPC):
            e = es0 + el
            wgu_np[el, 0] = qw(prep_stage1_w(gate_w[e].T))      # [H, F]
            wgu_np[el, 1] = qw(prep_stage1_w(up_w[e].T))
            wd_np[el] = qw(
                np.ascontiguousarray(
                    down_w[e].T.reshape(FO, 128, H).transpose(1, 0, 2)
                )
            )

        rsl = slice(c * FSH, (c + 1) * FSH)
        wsgu_np = np.stack(
            [prep_stage1_w(sg_w[rsl].T), prep_stage1_w(su_w[rsl].T)]
        ).astype(f16_np)
        wsd_np = np.ascontiguousarray(sd_w[:, rsl].T).astype(f16_np)

        in_maps.append(
            {
                "xeT": xeT_np,
                "wgu": wgu_np,
                "wd": wd_np,
                "xTr": xTr_np,
                "wsgu": wsgu_np,
                "wsd": wsd_np,
            }
        )

    nc = _get_nc()
    trace = bool(int(os.environ.get("KERNEL_TRACE", "0")))
    if trace:
        trace = _ensure_ntff_hook()
    for _ in range(int(os.environ.get("KERNEL_RUNS", "1"))):
        res = run_bass_kernel_spmd(
            nc, in_maps, core_ids=list(range(NCORES)), trace=trace
        )
    LAST_RESULTS["exec_time_ns"] = res.exec_time_ns
    LAST_RESULTS["mean_exec_time_ns"] = getattr(res, "mean_exec_time_ns", None)
    LAST_RESULTS["profile_json"] = res.profile_json
    LAST_RESULTS["insts_and_trace"] = res.instructions_and_trace
    LAST_RESULTS["raw"] = res.results

    # ---- combine on host ----
    ye_all = np.stack(
        [r["ye"] for r in res.results]
    ).reshape(E, CAPD, H).astype(np.float64)                      # [E, CAPD, H]
    ye_all /= YE_UNSCALE
    w_flat = topk_w.reshape(-1)
    y_entry = ye_all[flat_e, pos_flat] * w_flat[:, None]
    out = y_entry.reshape(T, K, H).sum(axis=1)

    for r in res.results:
        out += r["part"].reshape(T, H).astype(np.float64)

    return out.astype(np.float32)


# revision 39
# speedup vs baseline: 1.0816x; 1.0816x over previous
"""DeepseekV2 MoE layer on 8 Trainium2 NeuronCores.

Strategy (expert-parallel, matching the sharding hint):
  - Host: gate (softmax + top-6) in float64, stable dispatch by expert —
    bit-identical routing to the fp32 reference.
  - Device, per core c (SPMD, one program): 4 experts' GLU MLPs (fp8e4m3
    weights/activations, DoubleRow matmuls) on the gathered token buffer
    (capacity 128 >= observed max count 117), plus a 1/8 tensor-parallel
    shard of the shared-expert GLU in fp16 (FS 2816 -> 352, tight pack).
  - Host: weighted scatter-add combine + sum of shared partials (fp64).

Performance model (measured):
  - PE floor ~111 us/core (routed fp8-DR 70 + shared fp16 37 + transposes 4,
    all ~1 column/cycle @ 2 GHz).  DMA floor ~105 us (45.2 MB/core at the
    ~430 GB/s per-core streaming rate; chip aggregate saturates ~3.2 TB/s).
  - The previous 164-181 us came from coupling stalls: PSUM hit 8/8 banks
    during shared stage 1 (4 parallel accumulators), DMA-trigger engines
    (sync+scalar HWDGE) blocked head-of-line on buffer-recycle semaphores,
    and a 13-18 us serial tail (last expert s1->mul->transpose->s2->store).

Schedule in this version (measured ~159 us max-core / ~149 mean, vs
164-183 for the previous kernel):
  - Weight DMAs stream in 4-ho stage-1 tiles (ws1 pool 12 slots) and
    4/4/3-fc stage-2 tiles (ws2 pool 6 slots).  Triggers for expert e+1
    are drip-fed from inside expert e's body right AFTER each silu batch
    (~2-3 transfers per ring per phase).  Two hard-won rules:
      (a) a DMA trigger occupies its HWDGE engine until the ring has
          space (~2 outstanding transfers), so any batch of triggers
          queued on the scalar ring ahead of silus delays them at DMA
          pace (a 20+ us convoy);
      (b) pool depths are sized so every trigger's buffer-recycle
          semaphore is already satisfied when the engine reaches it.
    Finer (4-ho) tiles halve the stream-lag whiplash at the ws1 pool
    limit near expert boundaries.
  - Shared-expert compute is cut into 12 quanta (4 gate, 4 up+transpose,
    4 down+store) used as PE filler at expert phase boundaries; shared
    stage-1 accumulates per-token-block sequentially (1 PSUM bank at a
    time instead of 4, which used to peg PSUM at 8/8 banks).
  - Expert 3's stage-2 runs hn-major on its last weight tile so each
    psy's cast+store pipelines with the remaining matmuls (short tail).
  - ye outputs are stored as fp8e4m3 at scale 64 (adds 0.41% rel err in
    isolation; total 1.09% vs the 2% gate), shared weights tight-packed
    (352, no 384 pad): 45.2 MB/core vs 46.7 baseline.
"""

import os
import numpy as np

T, H, E, K = 512, 2048, 32, 6
F, FS = 1408, 2816
NCORES = 8
EPC = E // NCORES          # experts per core = 4
CAPD = 128                 # device per-expert capacity (max observed count 117)
CAP_REF = 160              # reference capacity (for drop semantics; no drops here)
HO = H // 128              # 16
FO = F // 128              # 11
TOK = T // 128             # 4
FSH = FS // NCORES         # 352 shared-intermediate shard (tight, no pad)
JT = [(0, 512), (512, 512), (1024, 384)]   # stage-1 f tiles
FS_CHUNKS = [(0, 128), (128, 128), (256, 96)]  # shared-intermediate chunks
S2_TILES = [(0, 4), (4, 4), (8, 3)]            # stage-2 f-chunk tiles

SILU_SCALE = 1.0 / 256.0   # fp8 psum -> h scale (w_scale 256 undone)
HT_SCALE = 1.0 / 16.0      # h -> hT fp8 scale
W_SCALE = 256.0            # expert weight quantization scale
W_CLIP = 224.0
YE_STORE_SCALE = 1.0 / 64.0   # psy (= 4096*y) -> fp8 store (= 64*y)
YE_UNSCALE = 64.0             # host divides stored ye by this
# NOTE: platform float8e4 is IEEE-style e4m3 with max 240 (not e4m3fn/448);
# max |64*y| ~ 130 leaves 1.8x headroom.

LAST_RESULTS = {}
_NC_CACHE = {}


def _build_nc():
    import concourse.tile as tile
    from concourse import mybir, bacc
    from concourse.masks import make_identity

    f32 = mybir.dt.float32
    f16 = mybir.dt.float16
    fp8 = mybir.dt.float8e4
    dr = mybir.MatmulPerfMode.DoubleRow

    nc = bacc.Bacc(None, target_bir_lowering=False, debug=False)

    xeT = nc.dram_tensor("xeT", [128, HO, EPC * CAPD], fp8, kind="ExternalInput")
    wgu = nc.dram_tensor("wgu", [EPC, 2, 128, HO, F], fp8, kind="ExternalInput")
    wd = nc.dram_tensor("wd", [EPC, 128, FO, H], fp8, kind="ExternalInput")
    xTr = nc.dram_tensor("xTr", [128, TOK, HO, 128], f16, kind="ExternalInput")
    wsgu = nc.dram_tensor("wsgu", [2, 128, HO, FSH], f16, kind="ExternalInput")
    wsd = nc.dram_tensor("wsd", [FSH, H], f16, kind="ExternalInput")
    ye = nc.dram_tensor("ye", [EPC, CAPD, H], fp8, kind="ExternalOutput")
    part = nc.dram_tensor("part", [TOK, 128, H], f16, kind="ExternalOutput")

    # Pool depths are sized so every weight-DMA trigger's buffer-recycle
    # semaphore is already satisfied when the trigger engine reaches it
    # (waves are emitted one expert ahead; ws1 holds ~1.75 experts of
    # stage-1 tiles, ws2 exactly 2 experts of stage-2 tiles).  A trigger
    # that waits head-of-line on the scalar ring delays the silus behind
    # it, which PE needs — that convoy was worth 20+ us/core.
    ws1_bufs = int(os.environ.get("KERNEL_WS1_BUFS", "12"))
    ws2_bufs = int(os.environ.get("KERNEL_WS2_BUFS", "6"))

    # Both HWDGE rings (sync + scalar) carry traffic round-robin.
    dma_engines = [nc.sync, nc.scalar]
    dma_i = [0]

    def dma(out_ap, in_ap):
        eng = dma_engines[dma_i[0] % 2]
        dma_i[0] += 1
        eng.dma_start(out_ap, in_ap)

    with tile.TileContext(nc) as tc:
        with (
            tc.tile_pool(name="res", bufs=2) as sb_res,
            tc.tile_pool(name="const", bufs=1) as sb_const,
            tc.tile_pool(name="ws1", bufs=ws1_bufs) as sb_w1,
            tc.tile_pool(name="ws2", bufs=ws2_bufs) as sb_w2,
            tc.tile_pool(name="wsh", bufs=4) as sb_wsh,
            tc.tile_pool(name="act", bufs=2) as sb_act,
            tc.tile_pool(name="oye", bufs=1) as sb_oye,
            tc.tile_pool(name="opart", bufs=1) as sb_opart,
            tc.tile_pool(name="acc", bufs=4, space="PSUM") as ps_acc,
            tc.tile_pool(name="py", bufs=4, space="PSUM") as ps_y,
        ):
            ident = sb_const.tile([128, 128], f16, tag="ident")
            make_identity(nc, ident)

            state = {}
            s1_tiles = {}
            s2_tiles = {}

            def issue_s1(e, proj, t):
                wt = sb_w1.tile([128, 4, F], fp8, tag="ws1",
                                name=f"w{e}_{proj}_{t}")
                dma(wt[:], wgu[e, proj, :, 4 * t: 4 * t + 4, :])
                s1_tiles[(e, proj, t)] = wt

            def issue_s2(e):
                lst = []
                for ti, (f0, fw) in enumerate(S2_TILES):
                    wt = sb_w2.tile([128, 4, H], fp8, tag="ws2",
                                    name=f"wd{e}_{ti}")
                    dma(wt[:, :fw, :], wd[e, :, f0: f0 + fw, :])
                    lst.append((f0, fw, wt))
                s2_tiles[e] = lst

            # ---------------- PRE: just enough to start e0 ----------------
            # Trigger cadence rule: a DMA trigger occupies its HWDGE engine
            # until the ring has space (~2 outstanding transfers), so any
            # batch of triggers queued on the scalar ring ahead of silus
            # delays them at DMA pace.  Keep <= ~5 transfers per ring ahead
            # of the next silu batch; later triggers are drip-fed from
            # inside the expert bodies right AFTER each silu batch.
            xeT_sb = sb_res.tile([128, HO, EPC * CAPD], fp8, tag="res",
                                 name="xeT_sb")
            xTr_sb = sb_res.tile([128, TOK, HO, 128], f16, tag="res",
                                 name="xTr_sb")
            issue_s1(0, 0, 0)
            dma(xeT_sb[:, 0:8, :], xeT[:, 0:8, :])
            issue_s1(0, 0, 1)
            issue_s1(0, 0, 2)
            wsg_tiles = []
            for t_ in range(2):
                wt = sb_wsh.tile([128, 8, FSH], f16, tag="wsh", name=f"wsg{t_}")
                dma(wt[:], wsgu[0, :, 8 * t_: 8 * t_ + 8, :])
                wsg_tiles.append(wt)
            state["wsg"] = wsg_tiles
            issue_s1(0, 0, 3)
            dma(xTr_sb[:, 0, :, :], xTr[:, 0, :, :])
            dma(xeT_sb[:, 8:16, :], xeT[:, 8:16, :])
            issue_s1(0, 1, 0)
            issue_s1(0, 1, 1)
            dma(xTr_sb[:, 1, :, :], xTr[:, 1, :, :])
            issue_s1(0, 1, 2)
            issue_s1(0, 1, 3)

            def issue_wsu():
                wsu_tiles = []
                for t_ in range(2):
                    wt = sb_wsh.tile([128, 8, FSH], f16, tag="wsh",
                                     name=f"wsu{t_}")
                    dma(wt[:], wsgu[1, :, 8 * t_: 8 * t_ + 8, :])
                    wsu_tiles.append(wt)
                state["wsu"] = wsu_tiles

            def issue_wsd():
                wsd_sb = sb_const.tile([128, 3, H], f16, tag="wsd",
                                       name="wsd_sb")
                dma(wsd_sb[:, 0, :], wsd[0:128, :])
                dma(wsd_sb[:, 1, :], wsd[128:256, :])
                dma(wsd_sb[:96, 2, :], wsd[256:352, :])
                state["wsd"] = wsd_sb

            def issue_xtr_tail():
                dma(xTr_sb[:, 2, :, :], xTr[:, 2, :, :])
                dma(xTr_sb[:, 3, :, :], xTr[:, 3, :, :])

            hs_all = sb_const.tile([128, TOK, FSH], f16, tag="hs", name="hs_all")
            hsT_all = sb_const.tile([128, TOK, 3, 128], f16, tag="hsT",
                                    name="hsT_all")

            # ---------------- shared-expert quanta ----------------
            def q_gate(tc_):
                def run():
                    psg = ps_acc.tile([128, FSH], f32, tag="acc",
                                      name=f"psg_{tc_}")
                    for t_ in range(2):
                        wt = state["wsg"][t_]
                        for hh in range(8):
                            ho = 8 * t_ + hh
                            nc.tensor.matmul(
                                psg[:],
                                xTr_sb[:, tc_, ho, :],
                                wt[:, hh, :],
                                start=(ho == 0),
                                stop=(ho == HO - 1),
                            )
                    nc.scalar.activation(
                        hs_all[:, tc_, :], psg[:],
                        mybir.ActivationFunctionType.Silu,
                    )
                return run

            def q_up(tc_):
                def run():
                    psu = ps_acc.tile([128, FSH], f32, tag="acc",
                                      name=f"psu_{tc_}")
                    for t_ in range(2):
                        wt = state["wsu"][t_]
                        for hh in range(8):
                            ho = 8 * t_ + hh
                            nc.tensor.matmul(
                                psu[:],
                                xTr_sb[:, tc_, ho, :],
                                wt[:, hh, :],
                                start=(ho == 0),
                                stop=(ho == HO - 1),
                            )
                    nc.vector.tensor_mul(
                        out=hs_all[:, tc_, :],
                        in0=hs_all[:, tc_, :],
                        in1=psu[:],
                    )
                    for c, (c0, cw) in enumerate(FS_CHUNKS):
                        pt = ps_y.tile([128, 512], f16, tag="py",
                                       name=f"pts_{tc_}_{c}")
                        nc.tensor.transpose(
                            pt[:cw, :128], hs_all[:, tc_, c0: c0 + cw], ident[:]
                        )
                        nc.vector.tensor_copy(
                            hsT_all[:cw, tc_, c, :], pt[:cw, :128]
                        )
                return run

            def q_down(tc_):
                def run():
                    wsd_sb = state["wsd"]
                    part_sb = sb_opart.tile([128, H], f16, tag="opart",
                                            name=f"part_{tc_}")
                    for hn in range(4):
                        psy = ps_y.tile([128, 512], f32, tag="py",
                                        name=f"pys_{tc_}_{hn}")
                        for c, (c0, cw) in enumerate(FS_CHUNKS):
                            nc.tensor.matmul(
                                psy[:],
                                hsT_all[:cw, tc_, c, :],
                                wsd_sb[:cw, c, hn * 512: (hn + 1) * 512],
                                start=(c == 0),
                                stop=(c == 2),
                            )
                        nc.vector.tensor_copy(
                            part_sb[:, hn * 512: (hn + 1) * 512], psy[:]
                        )
                    dma(part[tc_], part_sb[:])
                return run

            # ---------------- routed expert ----------------
            def expert(e, q_p0=(), q_p1=(), q_tr=(), split_out=False):
                esl = slice(e * CAPD, (e + 1) * CAPD)
                h_sb = sb_act.tile([128, F], f16, tag="h", name=f"h_{e}")
                for proj in range(2):
                    ps_j = [
                        ps_acc.tile([128, jw], f32, tag="acc",
                                    name=f"ps_{e}_{proj}_{j}")
                        for j, (j0, jw) in enumerate(JT)
                    ]
                    for t_ in range(4):
                        wt = s1_tiles.pop((e, proj, t_))
                        for hp in range(2):
                            ho = 4 * t_ + 2 * hp
                            for j, (j0, jw) in enumerate(JT):
                                nc.tensor.matmul(
                                    ps_j[j][:],
                                    xeT_sb[:, ho: ho + 2, esl],
                                    wt[:, 2 * hp: 2 * hp + 2, j0: j0 + jw],
                                    start=(ho == 0),
                                    stop=(ho == HO - 2),
                                    perf_mode=dr,
                                )
                    if proj == 0:
                        for j, (j0, jw) in enumerate(JT):
                            nc.scalar.activation(
                                h_sb[:, j0: j0 + jw],
                                ps_j[j][:],
                                mybir.ActivationFunctionType.Silu,
                                scale=SILU_SCALE,
                            )
                        for fn in q_p0:
                            fn()
                    else:
                        for j, (j0, jw) in enumerate(JT):
                            nc.vector.tensor_mul(
                                out=h_sb[:, j0: j0 + jw],
                                in0=h_sb[:, j0: j0 + jw],
                                in1=ps_j[j][:],
                            )
                        for fn in q_p1:
                            fn()

                hT_sb = sb_act.tile([128, FO, CAPD], fp8, tag="hT",
                                    name=f"hT_{e}")
                for fc in range(FO):
                    pt = ps_y.tile([128, 512], f16, tag="py",
                                   name=f"pt_{e}_{fc}")
                    nc.tensor.transpose(
                        pt[:, :128], h_sb[:, fc * 128: (fc + 1) * 128], ident[:]
                    )
                    nc.vector.tensor_scalar_mul(
                        hT_sb[:, fc, :], pt[:, :128], HT_SCALE
                    )
                for fn in q_tr:
                    fn()

                psy = [
                    ps_y.tile([128, 512], f32, tag="py", name=f"py_{e}_{hn}")
                    for hn in range(4)
                ]
                tiles = s2_tiles.pop(e)
                ye_sb = sb_oye.tile([128, H], fp8, tag="oye", name=f"ye_{e}")
                # last tile handled hn-major on the final expert so each
                # psy's cast+store pipelines with the remaining matmuls
                body, last = (tiles[:-1], tiles[-1]) if split_out else (tiles, None)
                for f0, fw, wt in body:
                    ff = 0
                    while ff < fw:
                        fc = f0 + ff
                        if ff + 1 < fw:
                            for hn in range(4):
                                nc.tensor.matmul(
                                    psy[hn][:],
                                    hT_sb[:, fc: fc + 2, :],
                                    wt[:, ff: ff + 2,
                                       hn * 512: (hn + 1) * 512],
                                    start=(fc == 0),
                                    stop=(fc + 2 == FO),
                                    perf_mode=dr,
                                )
                            ff += 2
                        else:
                            for hn in range(4):
                                nc.tensor.matmul(
                                    psy[hn][:],
                                    hT_sb[:, fc, :],
                                    wt[:, ff, hn * 512: (hn + 1) * 512],
                                    start=(fc == 0),
                                    stop=(fc == FO - 1),
                                )
                            ff += 1
                # On the last expert the casts alternate scalar (Copy
                # activation with scale) and vector so both engines drain
                # the 4 psy banks in parallel at the tail, where scalar is
                # otherwise idle.  Mid-kernel (e0-e2) they stay on vector —
                # a scalar cast there sits ahead of later silus in the
                # scalar FIFO and can delay them.
                def ye_cast(hn):
                    hsl = slice(hn * 512, (hn + 1) * 512)
                    if split_out and hn % 2 == 0:
                        nc.scalar.activation(
                            ye_sb[:, hsl], psy[hn][:],
                            mybir.ActivationFunctionType.Copy,
                            scale=YE_STORE_SCALE,
                        )
                    else:
                        nc.vector.tensor_scalar_mul(
                            ye_sb[:, hsl], psy[hn][:], YE_STORE_SCALE,
                        )
                    return hsl

                if split_out:
                    f0, fw, wt = last
                    for hn in range(4):
                        hsl = slice(hn * 512, (hn + 1) * 512)
                        nc.tensor.matmul(
                            psy[hn][:], hT_sb[:, f0: f0 + 2, :],
                            wt[:, 0:2, hsl], start=False, stop=False,
                            perf_mode=dr,
                        )
                        nc.tensor.matmul(
                            psy[hn][:], hT_sb[:, f0 + 2, :],
                            wt[:, 2, hsl], start=False, stop=True,
                        )
                        ye_cast(hn)
                        dma(ye[e][:, hsl], ye_sb[:, hsl])
                else:
                    for hn in range(4):
                        ye_cast(hn)
                    dma(ye[e], ye_sb[:])

            # ---------------- schedule ----------------
            # Each expert body drip-feeds the NEXT expert's weight triggers
            # right after its own silu batches (~3 transfers per ring per
            # phase), so the scalar ring never queues enough pending
            # transfers to delay a silu, and every trigger's buffer-recycle
            # semaphore is already satisfied when issued (pool depths hold
            # 1.5 experts of stage-1 and 2 of stage-2 tiles).
            def s1w(e, proj):
                def run():
                    for t_ in range(4):
                        issue_s1(e, proj, t_)
                return run

            def s2w(e):
                return lambda: issue_s2(e)

            expert(
                0,
                q_p0=[q_gate(0), s2w(0), issue_xtr_tail],
                q_p1=[q_gate(1), s1w(1, 0), lambda: issue_wsu()],
                q_tr=[q_gate(2), s1w(1, 1)],
            )
            expert(
                1,
                q_p0=[q_gate(3), s2w(1), issue_wsd],
                q_p1=[q_up(0), q_up(1), s1w(2, 0)],
                q_tr=[q_up(2), s1w(2, 1)],
            )
            expert(
                2,
                q_p0=[q_up(3), s2w(2)],
                q_p1=[q_down(0), s1w(3, 0)],
                q_tr=[q_down(1), s1w(3, 1)],
            )
            expert(
                3,
                q_p0=[q_down(2), s2w(3)],
                q_p1=[q_down(3)],
                split_out=True,
            )

    nc.finalize()
    return nc


def _get_nc():
    if "nc" not in _NC_CACHE:
        _NC_CACHE["nc"] = _build_nc()
    return _NC_CACHE["nc"]


def _ensure_ntff_hook():
    """Provide antenv.axon_hooks if the image lacks it (profiling only)."""
    try:
        from antenv.axon_hooks import get_axon_ntff_profile_hook  # noqa: F401

        return True
    except ImportError:
        pass
    try:
        import sys
        import types
        import ctypes
        import contextlib

        so_path = "/opt/axon/libaxon_pjrt.so"
        lib = ctypes.CDLL(so_path)
        if not hasattr(lib, "axon_start_nrt_profile"):
            return False
        lib.axon_start_nrt_profile.argtypes = [
            ctypes.POINTER(ctypes.c_int64),
            ctypes.c_size_t,
        ]
        lib.axon_start_nrt_profile.restype = ctypes.c_int64
        lib.axon_stop_nrt_profile.argtypes = [ctypes.c_char_p]
        lib.axon_stop_nrt_profile.restype = ctypes.c_int64

        @contextlib.contextmanager
        def _hook(output_dir, device_ids):
            import jax

            jax.devices()
            if device_ids:
                ids = (ctypes.c_int64 * len(device_ids))(*device_ids)
                rc = lib.axon_start_nrt_profile(ids, len(device_ids))
            else:
                rc = lib.axon_start_nrt_profile(None, 0)
            if rc != 0:
                raise RuntimeError(f"axon_start_nrt_profile rc={rc}")
            try:
                yield
            finally:
                n = lib.axon_stop_nrt_profile(str(output_dir).encode())
                print(f"ntff profile: {n} file(s) -> {output_dir}", file=sys.stderr)

        import antenv

        mod = types.ModuleType("antenv.axon_hooks")
        _holder = {"hook": _hook}
        mod.get_axon_ntff_profile_hook = lambda: _holder["hook"]

        def _set(h):
            _holder["hook"] = h

        mod.set_axon_ntff_profile_hook = _set
        sys.modules["antenv.axon_hooks"] = mod
        antenv.axon_hooks = mod
        return True
    except Exception:
        return False


def kernel(hidden_states, wg, gate_w, up_w, down_w, sg_w, su_w, sd_w):
    from concourse import mybir
    from concourse.bass_utils import run_bass_kernel_spmd

    x = np.asarray(hidden_states, np.float32)
    wg = np.asarray(wg, np.float32)
    gate_w = np.asarray(gate_w, np.float32)
    up_w = np.asarray(up_w, np.float32)
    down_w = np.asarray(down_w, np.float32)
    sg_w = np.asarray(sg_w, np.float32)
    su_w = np.asarray(su_w, np.float32)
    sd_w = np.asarray(sd_w, np.float32)

    # ---- gate: fp64 softmax + greedy top-k (matches fp32 reference routing;
    #      min 6th/7th margin ~2e-5 >> fp32 rounding noise) ----
    logits = x.astype(np.float64) @ wg.astype(np.float64).T
    m = logits.max(axis=-1, keepdims=True)
    es = np.exp(logits - m)
    scores = es / es.sum(axis=-1, keepdims=True)
    topk_idx = np.argsort(-scores, axis=-1, kind="stable")[:, :K]     # [T, K]
    topk_w = np.take_along_axis(scores, topk_idx, axis=-1)            # [T, K]

    # ---- dispatch: stable sort of (t, k) entries by expert ----
    N = T * K
    flat_e = topk_idx.reshape(-1)
    order = np.argsort(flat_e, kind="stable")
    sorted_e = flat_e[order]
    counts = np.bincount(flat_e, minlength=E)
    offsets = np.cumsum(counts) - counts
    pos_sorted = np.arange(N) - offsets[sorted_e]
    pos_flat = np.empty(N, np.int64)
    pos_flat[order] = pos_sorted
    tok_flat = np.arange(N) // K
    # reference drops entries with pos >= CAP_REF (none for this input);
    # device capacity is CAPD
    assert counts.max() <= CAPD, f"expert overflow: {counts.max()} > {CAPD}"

    buf = np.zeros((E, CAPD, H), np.float32)
    buf[flat_e, pos_flat] = x[tok_flat]

    fp8_np = np.dtype(mybir.dt.np(mybir.dt.float8e4))
    f16_np = np.dtype(np.float16)

    def qw(a):  # quantize an expert weight array to fp8 at scale 256
        a = np.clip(a * W_SCALE, -W_CLIP, W_CLIP)
        return np.ascontiguousarray(a).astype(fp8_np)

    def prep_stage1_w(w_t):  # w_t: [H, Fdim] -> [128, H//128, Fdim]
        fdim = w_t.shape[1]
        return np.ascontiguousarray(
            w_t.reshape(HO, 128, fdim).transpose(1, 0, 2)
        )

    xTr_np = np.ascontiguousarray(
        x.reshape(TOK, 128, HO, 128).transpose(3, 0, 2, 1)
    ).astype(f16_np)

    in_maps = []
    for c in range(NCORES):
        es0 = c * EPC
        xe_core = buf[es0: es0 + EPC].reshape(EPC * CAPD, H)  # [512, H]
        xeT_np = np.ascontiguousarray(
            xe_core.T.reshape(HO, 128, EPC * CAPD).transpose(1, 0, 2)
        ).astype(fp8_np)

        wgu_np = np.empty((EPC, 2, 128, HO, F), fp8_np)
        wd_np = np.empty((EPC, 128, FO, H), fp8_np)
        for el in range(EPC):
            e = es0 + el
            wgu_np[el, 0] = qw(prep_stage1_w(gate_w[e].T))      # [H, F]
            wgu_np[el, 1] = qw(prep_stage1_w(up_w[e].T))
            wd_np[el] = qw(
                np.ascontiguousarray(
                    down_w[e].T.reshape(FO, 128, H).transpose(1, 0, 2)
                )
            )

        rsl = slice(c * FSH, (c + 1) * FSH)
        wsgu_np = np.stack(
            [prep_stage1_w(sg_w[rsl].T), prep_stage1_w(su_w[rsl].T)]
        ).astype(f16_np)
        wsd_np = np.ascontiguousarray(sd_w[:, rsl].T).astype(f16_np)

        in_maps.append(
            {
                "xeT": xeT_np,
                "wgu": wgu_np,
                "wd": wd_np,
                "xTr": xTr_np,
                "wsgu": wsgu_np,
                "wsd": wsd_np,
            }
        )

    nc = _get_nc()
    trace = bool(int(os.environ.get("KERNEL_TRACE", "0")))
    if trace:
        trace = _ensure_ntff_hook()
    for _ in range(int(os.environ.get("KERNEL_RUNS", "1"))):
        res = run_bass_kernel_spmd(
            nc, in_maps, core_ids=list(range(NCORES)), trace=trace
        )
    LAST_RESULTS["exec_time_ns"] = res.exec_time_ns
    LAST_RESULTS["mean_exec_time_ns"] = getattr(res, "mean_exec_time_ns", None)
    LAST_RESULTS["profile_json"] = res.profile_json
    LAST_RESULTS["insts_and_trace"] = res.instructions_and_trace
    LAST_RESULTS["raw"] = res.results

    # ---- combine on host ----
    ye_all = np.stack(
        [r["ye"] for r in res.results]
    ).reshape(E, CAPD, H).astype(np.float64)                      # [E, CAPD, H]
    ye_all /= YE_UNSCALE
    w_flat = topk_w.reshape(-1)
    y_entry = ye_all[flat_e, pos_flat] * w_flat[:, None]
    out = y_entry.reshape(T, K, H).sum(axis=1)

    for r in res.results:
        out += r["part"].reshape(T, H).astype(np.float64)

    return out.astype(np.float32)
